# revision 59
# baseline (speedup 1.0000x reference)
"""Trainium2 Bass kernel for nn_Net_84782654423525 (GNN message passing + LSTM).

Strategy (8 NeuronCores, dst-sharded nodes):
  Launch A (mpnn1): per core per timestep, gather X[src] for edges whose dst
    it owns via non-transposed HBM dma_gather into an int32-typed tile (the
    gather is priced by output free size, so packing the 256-byte token as
    64xint32 instead of 128xfp16 halves its cost; int64 scrambles data on
    the real SWDGE path).  Consumers read the tile through bitcast-fp16
    access patterns.  The gathered layout is node-major: [dst-lane
    (partition), slab, feature].
    Degree-sorted node groups with a uniform slab count per call let the
    whole call fold as one strided DVE add per tree level; a tunable share
    of groups folds on PE instead (identity-matmul accumulate into PSUM,
    evicted by Act with fused Relu).  Output h1 = relu(segment_sum) only;
    the mean divide (alpha) and BatchNorm are per-node / per-feature affines
    folded into the host-side repack between launches.
  Host: unpermute h1 rows, apply alpha + BN1 in fp32, build the launch-B
    gather table (int32-packed) and the feature-major LSTM input tiles.
  Launch B (mpnn2 + 2-layer LSTM + dense): same gather/fold on the h1
    table -> raw h2 strip (node-major); relu on DVE; unpermute+transpose to
    feature-major natural order in one SBUF-source dma_gather (strip rows
    are tokens); BN2 as a single Act op (per-partition scale/bias in
    feature-major).  LSTM over [h1;h2] with fp16 matmuls, fp32 PSUM, gate
    activations merged in 1024-wide pairs, layer-2 steps interleaved one
    node-tile behind layer 1; final dense + ReLU.

fp16 for gathers/matmuls/elementwise keeps DVE 2x throughput and end-to-end
rel err at ~5e-3.  f32 where cheap (PSUM accumulation, host-side BN).
"""

import os
import sys
from contextlib import ExitStack

import numpy as np

sys.path.insert(0, "/opt/trn_rl_repo")

import concourse.bacc as bacc
import concourse.tile as tile
from concourse import mybir
from concourse.bass_utils import run_bass_kernel_spmd

HDT = mybir.dt.float16
F32 = mybir.dt.float32
I16 = mybir.dt.int16
I32 = mybir.dt.int32
AF = mybir.ActivationFunctionType
EPS = 1e-3
NCORES = 8
CALL_CAP_A = 6400   # smaller calls -> tighter uniform-K padding (A: Pool-bound)
CALL_CAP_B = 12800  # fewer DVE tree-op overheads (B: DVE/PE-bound)

PROFILE = bool(int(os.environ.get("KERNEL_PROFILE", "0")))
LAST_STATS = {}

try:  # trace=True requires antenv.axon_hooks; fall back gracefully
    from antenv.axon_hooks import get_axon_ntff_profile_hook  # noqa: F401
except Exception:
    PROFILE = False


# ---------------------------------------------------------------- host prep

def _pack_idx(stream):
    """idx i of the stream lives at [i % 16, i // 16]; tiled to 128 rows."""
    n = len(stream)
    m = stream.reshape(n // 16, 16).T
    return np.ascontiguousarray(np.tile(m, (8, 1))).astype(np.int16)


def _plan_t(src, dst, n, ncores, shp, pad_tok, call_cap):
    """Edge plan for one timestep: degree-sorted node groups, gather calls
    with a uniform slab count K per call.

    Returns (calls [(g0, g1, Kc)], streams per core, alpha per core
    (permuted order), perm per core)."""
    sh = n // ncores
    ng = shp // 128
    per_core = []
    for c in range(ncores):
        m = (dst >= c * sh) & (dst < (c + 1) * sh)
        dl = (dst[m] - c * sh).astype(np.int64)
        sl = src[m].astype(np.int64)
        order = np.argsort(dl, kind="stable")
        dl = dl[order]
        sl = sl[order]
        cnt = np.bincount(dl, minlength=sh)
        perm = np.argsort(-cnt, kind="stable")  # natural ids, deg-desc order
        pos_of = np.empty(sh, np.int64)
        pos_of[perm] = np.arange(sh)
        per_core.append((dl, sl, cnt, perm, pos_of))
    K = np.ones(ng, np.int64)
    for dl, sl, cnt, perm, pos_of in per_core:
        cp = np.zeros(shp, np.int64)
        cp[:sh] = cnt[perm]
        K = np.maximum(K, cp.reshape(ng, 128).max(1))
    # contiguous calls, slab count uniform at the call's max (K non-increasing)
    calls = []
    g0 = 0
    while g0 < ng:
        Kc = int(K[g0])
        g1 = g0 + 1
        while g1 < ng and 128 * Kc * (g1 + 1 - g0) <= call_cap:
            g1 += 1
        calls.append((g0, g1, Kc))
        g0 = g1
    # split the final group into its own small call: the unpermute (and the
    # next timestep) waits on the whole strip, so a short last fold chain
    # shortens the per-timestep tail
    if calls and calls[-1][1] - calls[-1][0] > 1:
        g0, g1, Kc = calls.pop()
        calls.append((g0, g1 - 1, Kc))
        calls.append((g1 - 1, g1, int(K[g1 - 1])))
    base_of_g = np.zeros(ng + 1, np.int64)
    kc_of_g = np.zeros(ng, np.int64)
    off = 0
    for (g0, g1, Kc) in calls:
        for g in range(g0, g1):
            base_of_g[g] = off + (g - g0) * 128 * Kc
            kc_of_g[g] = Kc
        off += 128 * Kc * (g1 - g0)
    L = int(off)
    streams, alphas, perms = [], [], []
    for dl, sl, cnt, perm, pos_of in per_core:
        stream = np.full(L, pad_tok, np.int64)
        starts = np.concatenate([[0], np.cumsum(cnt)])
        j = np.arange(dl.size) - starts[dl]
        p = pos_of[dl]  # permuted position of each edge's dst
        pos = base_of_g[p // 128] + j * 128 + (p % 128)
        stream[pos] = sl
        streams.append(stream)
        a = np.ones(shp, np.float32)
        a[:sh] = 1.0 / np.maximum(cnt[perm], 1.0)
        alphas.append(a)
        perms.append(perm)
    return calls, streams, alphas, perms


# ---------------------------------------------------------- device builders

def _tree_fold(eng, gvr, sv, Kc):
    """Uniform-K pairwise fold of gvr [128, G', Kc, 128] on engine handle
    `eng`; final level writes sv (strip view [128, G', 128])."""
    kk = Kc
    while kk > 2:
        h = kk // 2
        rem = kk - h
        eng.tensor_add(gvr[:, :, 0:h, :], gvr[:, :, 0:h, :],
                       gvr[:, :, rem:kk, :])
        kk = rem
    if kk == 2:
        eng.tensor_add(sv, gvr[:, :, 0, :], gvr[:, :, 1, :])
    else:
        eng.tensor_copy(sv, gvr[:, :, 0, :])


def _emit_mpnn(nc, pools, src_ap, idx_d, idx_off, calls, strip, identt,
               fr_pe, fr_pool, steps=(), out_cb=None):
    """Packed gather + per-call uniform-K fold for one timestep into strip
    [128, shp] fp16 (node-major, raw relu'd sums).  Groups are assigned
    [PE | Pool | DVE] per call, greedily tracking the slot-share targets
    fr_pe / fr_pool.  After each call, emits the next closure from `steps`
    (previous timestep's LSTM node-tile steps).  Returns new idx_off."""
    si = 0
    tot = pe_tot = pool_tot = 0
    for (g0, g1, Kc) in calls:
        G = g1 - g0
        Lc = 128 * Kc * G
        idxt = pools["idx"].tile([128, Lc // 16], I16, tag="idx")
        nc.sync.dma_start(
            idxt[:], idx_d.ap()[:, idx_off // 16 : (idx_off + Lc) // 16])
        gt = pools["g"].tile([128, G * Kc, 64], I32, tag="g")
        nc.gpsimd.dma_gather(gt[:], src_ap, idxt[:], Lc, Lc, 64,
                             transpose=False, single_packet=False)
        gv = gt[:].bitcast(HDT).rearrange("p (g k) e -> p g k e", g=G)
        gsl = 128 * Kc
        tot += Lc
        npe = npool = 0
        for _ in range(G):
            if (pe_tot + 0.5 * gsl) <= fr_pe * tot and npe + npool < G:
                pe_tot += gsl
                npe += 1
            elif (pool_tot + 0.5 * gsl) <= fr_pool * tot and npe + npool < G:
                pool_tot += gsl
                npool += 1
        for gi in range(npe):
            pst = pools["psf"].tile([128, 128], F32, tag="psf")
            for j in range(Kc):
                nc.tensor.matmul(pst[:], identt[:], gv[:, gi, j, :],
                                 start=(j == 0), stop=(j == Kc - 1))
            gcol = (g0 + gi) * 128
            nc.scalar.activation(strip[:, gcol : gcol + 128], pst[:], AF.Relu)
        if npool:
            sl0 = (g0 + npe) * 128
            sl1 = (g0 + npe + npool) * 128
            sv = strip[:, sl0:sl1].rearrange("p (g e) -> p g e", g=npool)
            _tree_fold(nc.gpsimd, gv[:, npe : npe + npool], sv, Kc)
        nd = G - npe - npool
        if nd:
            sl0 = (g0 + npe + npool) * 128
            sv = strip[:, sl0 : g1 * 128].rearrange("p (g e) -> p g e", g=nd)
            _tree_fold(nc.vector, gv[:, npe + npool : G], sv, Kc)
        if npool or nd:
            # relu the Pool+DVE ranges (contiguous) in one 4x DVE op
            sl0 = (g0 + npe) * 128
            nc.vector.tensor_scalar_max(strip[:, sl0 : g1 * 128],
                                        strip[:, sl0 : g1 * 128], 0.0)
        if out_cb is not None:
            out_cb(g0 * 128, g1 * 128)
        idx_off += Lc
        if si < len(steps):
            steps[si]()
            si += 1
    while si < len(steps):
        steps[si]()
        si += 1
    return idx_off


def _build_launch_a(calls_all, w, ntok, shp, fr_pe, fr_pool):
    nc = bacc.Bacc("TRN2", target_bir_lowering=False, debug=False,
                   num_devices=NCORES)
    Ltot = int(sum(128 * Kc * (g1 - g0) for calls in calls_all
                   for (g0, g1, Kc) in calls))
    xf_d = nc.dram_tensor("xf", [w, ntok, 64], I32, kind="ExternalInput")
    idx_d = nc.dram_tensor("idx", [128, Ltot // 16], I16, kind="ExternalInput")
    ident_d = nc.dram_tensor("ident", [128, 128], HDT, kind="ExternalInput")
    h1_d = nc.dram_tensor("h1", [w, 128, shp], HDT, kind="ExternalOutput")

    with tile.TileContext(nc) as tc, ExitStack() as ctx, \
            nc.allow_low_precision(reason="fp16 fold tree by design"):
        pools = {
            "idx": ctx.enter_context(tc.tile_pool(name="idx", bufs=8)),
            "g": ctx.enter_context(tc.tile_pool(name="g", bufs=4)),
            "strip": ctx.enter_context(tc.tile_pool(name="strip", bufs=2)),
            "w": ctx.enter_context(tc.tile_pool(name="w", bufs=1)),
            "psf": ctx.enter_context(tc.tile_pool(name="psf", bufs=8,
                                                  space="PSUM")),
        }
        identt = pools["w"].tile([128, 128], HDT, tag="ident")
        nc.sync.dma_start(identt[:], ident_d.ap()[:])
        idx_off = 0
        for t in range(w):
            strip = pools["strip"].tile([128, shp], HDT, tag="strip")

            def _out(c0, c1, t=t, strip=strip):
                # per-call writeback: the launch tail only waits for the
                # small final call instead of the whole strip
                nc.sync.dma_start(h1_d.ap()[t][:, c0:c1], strip[:, c0:c1])

            idx_off = _emit_mpnn(nc, pools, xf_d.ap()[t], idx_d, idx_off,
                                 calls_all[t], strip, identt, fr_pe, fr_pool,
                                 out_cb=_out)
    nc.compile()
    return nc


def _lstm_step_nt(nc, pools, xa, xb, ka, kb, ra, rb_, h, c, first, ct, nt):
    """One LSTM node-tile step, gate-pair-merged (biases all-zero; asserted
    on host).  xa/xb: fn(nt) -> AP [128, ct] input halves.  h/c: [128,
    ntile, 2*ct] fp16 tiles updated in place."""
    pairs = []
    for pair in range(4):  # keras gate pairs: i, f, g(cell), o
        if first and pair == 1:
            pairs.append(None)  # f gate multiplies c(-1)=0: skip at t0
            continue
        ps = pools["psum2"].tile([128, 2 * ct], F32, tag="ps")
        for half in range(2):
            gs = slice((2 * pair + half) * 128, (2 * pair + half + 1) * 128)
            o_ap = ps[:, half * ct : (half + 1) * ct]
            if first:
                nc.tensor.matmul(o_ap, ka[:, gs], xa(nt), start=True,
                                 stop=False)
                nc.tensor.matmul(o_ap, kb[:, gs], xb(nt), start=False,
                                 stop=True)
            else:
                # recurrent contribution first: h(t-1) is ready long before
                # this timestep's x (h2n), so PE can run these while the
                # strip/unpermute chain is still in flight
                nc.tensor.matmul(o_ap, ra[:, gs], h[:, nt, 0:ct],
                                 start=True, stop=False)
                nc.tensor.matmul(o_ap, rb_[:, gs], h[:, nt, ct : 2 * ct],
                                 start=False, stop=False)
                nc.tensor.matmul(o_ap, ka[:, gs], xa(nt), start=False,
                                 stop=False)
                nc.tensor.matmul(o_ap, kb[:, gs], xb(nt), start=False,
                                 stop=True)
        gt_ = pools["gate"].tile([128, 2 * ct], HDT, tag="gate")
        func = AF.Tanh if pair == 2 else AF.Sigmoid
        nc.scalar.activation(gt_[:], ps[:], func)
        pairs.append(gt_)
    i_, f_, g_, o_ = pairs
    csl = c[:, nt, :]
    tmp = pools["tmp"].tile([128, 2 * ct], HDT, tag="tmp")
    nc.vector.tensor_mul(tmp[:], i_[:], g_[:])
    if first:
        nc.vector.tensor_copy(csl, tmp[:])
    else:
        nc.vector.tensor_mul(csl, f_[:], csl)
        nc.vector.tensor_add(csl, csl, tmp[:])
    th = pools["tmp"].tile([128, 2 * ct], HDT, tag="tmp")
    nc.scalar.activation(th[:], csl, AF.Tanh)
    nc.vector.tensor_mul(h[:, nt, :], o_[:], th[:])


def _build_launch_b(calls_all, w, f, ntok, shp, u4, fr_pe, fr_pool):
    nc = bacc.Bacc("TRN2", target_bir_lowering=False, debug=False,
                   num_devices=NCORES)
    Ltot = int(sum(128 * Kc * (g1 - g0) for calls in calls_all
                   for (g0, g1, Kc) in calls))
    ct = 512
    ntile = shp // ct
    hf_d = nc.dram_tensor("hf", [w, ntok, 64], I32, kind="ExternalInput")
    idx_d = nc.dram_tensor("idx", [128, Ltot // 16], I16, kind="ExternalInput")
    h1t_d = nc.dram_tensor("h1t", [w, 128, shp], HDT, kind="ExternalInput")
    rsg_d = nc.dram_tensor("rsg2", [w, 128, 1], F32, kind="ExternalInput")
    bet_d = nc.dram_tensor("bet2", [w, 128, 1], F32, kind="ExternalInput")
    k1_d = nc.dram_tensor("k1", [256, u4], HDT, kind="ExternalInput")
    r1_d = nc.dram_tensor("r1", [256, u4], HDT, kind="ExternalInput")
    k2_d = nc.dram_tensor("k2", [256, u4], HDT, kind="ExternalInput")
    r2_d = nc.dram_tensor("r2", [256, u4], HDT, kind="ExternalInput")
    wd_d = nc.dram_tensor("wd", [128, 2], HDT, kind="ExternalInput")
    bd_d = nc.dram_tensor("bd", [1, 1], F32, kind="ExternalInput")
    pidx_d = nc.dram_tensor("pinv", [w, 128, shp // 16], I16,
                            kind="ExternalInput")
    ident_d = nc.dram_tensor("ident", [128, 128], HDT, kind="ExternalInput")
    y_d = nc.dram_tensor("y", [1, shp], F32, kind="ExternalOutput")

    with tile.TileContext(nc) as tc, ExitStack() as ctx, \
            nc.allow_low_precision(reason="fp16 state/fold by design"):
        pools = {
            "idx": ctx.enter_context(tc.tile_pool(name="idx", bufs=6)),
            "g": ctx.enter_context(tc.tile_pool(name="g", bufs=3)),
            "strip": ctx.enter_context(tc.tile_pool(name="strip", bufs=2)),
            "misc": ctx.enter_context(tc.tile_pool(name="misc", bufs=2)),
            "w": ctx.enter_context(tc.tile_pool(name="w", bufs=1)),
            "state": ctx.enter_context(tc.tile_pool(name="state", bufs=1)),
            "gate": ctx.enter_context(tc.tile_pool(name="gate", bufs=8)),
            "tmp": ctx.enter_context(tc.tile_pool(name="tmp", bufs=4)),
            "yd": ctx.enter_context(tc.tile_pool(name="yd", bufs=2)),
            "h1t": ctx.enter_context(tc.tile_pool(name="h1t", bufs=2)),
            "h2n": ctx.enter_context(tc.tile_pool(name="h2n", bufs=2)),
            "psum2": ctx.enter_context(tc.tile_pool(name="psum2", bufs=3,
                                                    space="PSUM")),
            "psf": ctx.enter_context(tc.tile_pool(name="psf", bufs=1,
                                                  space="PSUM")),
            "psd": ctx.enter_context(tc.tile_pool(name="psd", bufs=1,
                                                  space="PSUM")),
        }
        # persistent weights: loaded during timestep 0's gathers so the
        # first idx DMA isn't queued behind them on SP
        wt = {}

        def _load_weights():
            for nm, d in (("k1", k1_d), ("r1", r1_d), ("k2", k2_d),
                          ("r2", r2_d)):
                for half in range(2):
                    tw = pools["w"].tile([128, u4], HDT, tag=f"{nm}{half}")
                    nc.sync.dma_start(tw[:],
                                      d.ap()[half * 128 : (half + 1) * 128])
                    wt[f"{nm}{half}"] = tw
            wdt = pools["w"].tile([128, 2], HDT, tag="wd")
            nc.sync.dma_start(wdt[:], wd_d.ap()[:])
            wt["wd"] = wdt
            bdt = pools["w"].tile([1, 1], F32, tag="bd")
            nc.sync.dma_start(bdt[:], bd_d.ap()[:])
            wt["bd"] = bdt

        identt = pools["w"].tile([128, 128], HDT, tag="ident")
        nc.sync.dma_start(identt[:], ident_d.ap()[:])

        # LSTM state: h and c for both layers, [128, ntile, 2*ct] fp16
        h1s = pools["state"].tile([128, ntile, 2 * ct], HDT, tag="h1s")
        c1s = pools["state"].tile([128, ntile, 2 * ct], HDT, tag="c1s")
        h2s = pools["state"].tile([128, ntile, 2 * ct], HDT, tag="h2s")
        c2s = pools["state"].tile([128, ntile, 2 * ct], HDT, tag="c2s")

        idx_off = 0
        prev_steps = []
        for t in range(w):
            strip = pools["strip"].tile([128, shp], HDT, tag="strip")
            idx_off = _emit_mpnn(nc, pools, hf_d.ap()[t], idx_d, idx_off,
                                 calls_all[t], strip, identt, fr_pe[t],
                                 fr_pool[t], steps=prev_steps)
            if t == 0:
                _load_weights()
            rsgt = pools["misc"].tile([128, 1], F32, tag="rsg")
            nc.sync.dma_start(rsgt[:], rsg_d.ap()[t])
            bett = pools["misc"].tile([128, 1], F32, tag="bet")
            nc.sync.dma_start(bett[:], bet_d.ap()[t])
            h1b = pools["h1t"].tile([128, shp], HDT, tag="h1t")
            nc.sync.dma_start(h1b[:], h1t_d.ap()[t])
            pit = pools["misc"].tile([128, shp // 16], I16, tag="pid")
            nc.sync.dma_start(pit[:], pidx_d.ap()[t])
            # strip -> h2n chain runs at high priority: it gates this
            # timestep's LSTM (emitted during the next timestep's calls).
            # unpermute h2 to natural token order + transpose to
            # feature-major in one SBUF-source gather (strip rows are
            # tokens), then BN2 as per-partition affine in feature-major.
            # High priority: the gather must jump ahead of the next
            # timestep's pool-share folds in the Pool queue, since the LSTM
            # epoch it gates is the wall-clock chain.
            h2n = pools["h2n"].tile([128, 1, shp], HDT, tag="h2n")
            nc.gpsimd.dma_gather(
                h2n[:], strip[:], pit[:], shp, shp, f, transpose=True,
                sbuf_tokens_per_rank=128, sbuf_free_dim_per_rank=2 * f,
                single_packet=False)
            nc.scalar.activation(h2n[:, 0, :], h2n[:, 0, :], AF.Identity,
                                 bias=bett[:], scale=rsgt[:])

            # LSTM steps for this timestep: emitted during the NEXT
            # timestep's gather calls (interleaved), so queue order lets
            # that timestep's folds run ahead of this LSTM's tail.
            def _mk_steps(t0, hb, hn):
                x1a = lambda nt: hb[:, nt * ct : (nt + 1) * ct]
                x1b = lambda nt: hn[:, 0, nt * ct : (nt + 1) * ct]
                x2a = lambda nt: h1s[:, nt, 0:ct]
                x2b = lambda nt: h1s[:, nt, ct : 2 * ct]

                def l1(k):
                    _lstm_step_nt(nc, pools, x1a, x1b, wt["k10"][:],
                                  wt["k11"][:], wt["r10"][:],
                                  wt["r11"][:], h1s, c1s, t0 == 0, ct, k)

                def l2(k):
                    _lstm_step_nt(nc, pools, x2a, x2b, wt["k20"][:],
                                  wt["k21"][:], wt["r20"][:],
                                  wt["r21"][:], h2s, c2s, t0 == 0, ct, k)

                def step(k):
                    if k < ntile:
                        l1(k)
                    if k >= 1:
                        l2(k - 1)

                if t0 == 0:
                    # t0 layer-1 steps are independent across node-tiles
                    # (no recurrent input): run them all before layer 2 to
                    # keep PE fed instead of chaining L1/L2 per tile
                    return ([(lambda k=k: l1(k)) for k in range(ntile)]
                            + [(lambda k=k: l2(k)) for k in range(ntile)])
                return [(lambda k=k: step(k)) for k in range(ntile + 1)]

            prev_steps = _mk_steps(t, h1b, h2n)

        def dense(nt):
            # y = relu(hT @ wd + bd) for one node-tile
            ps = pools["psd"].tile([1, ct], F32, tag="psy")
            nc.tensor.matmul(ps[:], wt["wd"][:, 0:1], h2s[:, nt, 0:ct],
                             start=True, stop=False)
            nc.tensor.matmul(ps[:], wt["wd"][:, 1:2], h2s[:, nt, ct : 2 * ct],
                             start=False, stop=True)
            yt = pools["yd"].tile([1, ct], F32, tag="y")
            nc.scalar.activation(yt[:], ps[:], AF.Relu,
                                 bias=wt["bd"][:, 0:1])
            nc.sync.dma_start(y_d.ap()[:, nt * ct : (nt + 1) * ct], yt[:])

        # final drain: t5's steps, dense head interleaved as each h2s
        # node-tile becomes final
        for k, s in enumerate(prev_steps):
            s()
            if k >= 1:
                dense(k - 1)
    nc.compile()
    return nc


# ----------------------------------------------------------------- kernel()

FR_PE_A, FR_POOL_A = 0.50, 0.0  # fold slot shares (launch A: Pool-bound)
# launch B: PE is busy with the LSTM except during t0's fold phase
FR_PE_B = [0.40, 0.18, 0.0, 0.0, 0.0, 0.0]
FR_POOL_B = [0.0, 0.14, 0.19, 0.19, 0.19, 0.19]


def kernel(**inputs):
    X = np.asarray(inputs["X"], np.float32)
    edge_src = np.asarray(inputs["edge_src"])
    edge_dst = np.asarray(inputs["edge_dst"])
    w, n, f = X.shape
    u4 = int(np.asarray(inputs["k1"]).shape[1])
    sh = n // NCORES
    ng = max(1, (sh + 127) // 128)
    shp = ng * 128
    ntok = n + 1
    pad_tok = n

    # fold BN params
    rsg1 = (np.asarray(inputs["gamma1"], np.float32)
            / np.sqrt(np.asarray(inputs["var1"], np.float32) + EPS))
    bet1 = (np.asarray(inputs["beta1"], np.float32)
            - np.asarray(inputs["mean1"], np.float32) * rsg1)
    rsg2 = (np.asarray(inputs["gamma2"], np.float32)
            / np.sqrt(np.asarray(inputs["var2"], np.float32) + EPS))
    bet2 = (np.asarray(inputs["beta2"], np.float32)
            - np.asarray(inputs["mean2"], np.float32) * rsg2)

    assert np.all(np.asarray(inputs["b1"]) == 0) and \
        np.all(np.asarray(inputs["b2"]) == 0), "nonzero LSTM bias unsupported"

    # edge plans (separate call partitioning per launch)
    calls_a, streams_a, alphas_a, perms_a = [], [], [], []
    calls_b, streams_b, perms_b = [], [], []
    for t in range(w):
        calls, streams, alphas, perms = _plan_t(np.asarray(edge_src[t]),
                                                np.asarray(edge_dst[t]),
                                                n, NCORES, shp, pad_tok,
                                                CALL_CAP_A)
        calls_a.append(calls)
        streams_a.append(streams)
        alphas_a.append(alphas)
        perms_a.append(perms)
        calls, streams, alphas, perms = _plan_t(np.asarray(edge_src[t]),
                                                np.asarray(edge_dst[t]),
                                                n, NCORES, shp, pad_tok,
                                                CALL_CAP_B)
        calls_b.append(calls)
        streams_b.append(streams)
        perms_b.append(perms)

    # packed inputs
    xf = np.zeros((w, ntok, f), np.float16)
    xf[:, :n] = X.astype(np.float16)
    xf_i64 = xf.reshape(w, ntok, f).view(np.int32)  # [w, ntok, 64]
    idx_packed_a, idx_packed_b = [], []
    for c in range(NCORES):
        idx_packed_a.append(np.concatenate(
            [_pack_idx(streams_a[t][c]) for t in range(w)], axis=1))
        idx_packed_b.append(np.concatenate(
            [_pack_idx(streams_b[t][c]) for t in range(w)], axis=1))
    ident = np.eye(128, dtype=np.float16)

    # ---- launch A
    nc_a = _build_launch_a(calls_a, w, ntok, shp, FR_PE_A, FR_POOL_A)
    in_maps_a = [
        dict(xf=xf_i64, idx=idx_packed_a[c], ident=ident)
        for c in range(NCORES)
    ]
    LAST_STATS["nc_a"] = nc_a
    res_a = run_bass_kernel_spmd(nc_a, in_maps_a, core_ids=list(range(NCORES)),
                                 trace=PROFILE)
    LAST_STATS["a_exec_ns"] = res_a.exec_time_ns
    h1_shards = [res_a.results[c]["h1"] for c in range(NCORES)]  # [w,128,shp]

    # ---- host exchange: unpermute rows, apply alpha + BN1 in fp32
    h1_full = np.empty((w, n, f), np.float32)
    for c in range(NCORES):
        # strip col (g, e), partition lane -> permuted position g*128+lane
        arr = h1_shards[c].reshape(w, 128, ng, f).transpose(0, 2, 1, 3)
        arr = arr.reshape(w, shp, f)[:, :sh].astype(np.float32)
        for t in range(w):
            av = alphas_a[t][c][:sh]
            h1_full[t, c * sh + perms_a[t][c]] = arr[t] * av[:, None]
    h1_full = h1_full * rsg1[:, None, :] + bet1[:, None, :]
    hf = np.zeros((w, ntok, f), np.float16)
    hf[:, :n] = h1_full.astype(np.float16)
    hf_i64 = hf.view(np.int32)
    h1t = []
    for c in range(NCORES):
        v = np.zeros((w, 128, shp), np.float16)
        v[:, :, :sh] = h1_full[:, c * sh : (c + 1) * sh, :].transpose(0, 2, 1)
        h1t.append(v)
    # inverse-permutation gather indices for launch B's h2 unpermute
    pinv_packed = []
    for c in range(NCORES):
        blocks = []
        for t in range(w):
            pos_of = np.zeros(shp, np.int64)
            pos_of[perms_b[t][c]] = np.arange(sh)
            blocks.append(_pack_idx(pos_of))
        pinv_packed.append(np.stack(blocks))

    # ---- launch B
    k1 = np.asarray(inputs["k1"], np.float32).astype(np.float16)
    r1 = np.asarray(inputs["r1"], np.float32).astype(np.float16)
    k2 = np.asarray(inputs["k2"], np.float32).astype(np.float16)
    r2 = np.asarray(inputs["r2"], np.float32).astype(np.float16)
    wd = np.asarray(inputs["wd"], np.float32).reshape(2, 128).T.copy().astype(
        np.float16)
    bd = np.asarray(inputs["bd"], np.float32).reshape(1, 1)
    rsg2_in = rsg2.reshape(w, 128, 1)
    bet2_in = bet2.reshape(w, 128, 1)

    nc_b = _build_launch_b(calls_b, w, f, ntok, shp, u4, FR_PE_B, FR_POOL_B)
    in_maps_b = [
        dict(hf=hf_i64, idx=idx_packed_b[c], h1t=h1t[c], rsg2=rsg2_in,
             bet2=bet2_in, k1=k1, r1=r1, k2=k2, r2=r2, wd=wd,
             bd=bd, pinv=pinv_packed[c], ident=ident)
        for c in range(NCORES)
    ]
    LAST_STATS["nc_b"] = nc_b
    res_b = run_bass_kernel_spmd(nc_b, in_maps_b, core_ids=list(range(NCORES)),
                                 trace=PROFILE)
    LAST_STATS["b_exec_ns"] = res_b.exec_time_ns

    out = np.empty((n, 1), np.float32)
    for c in range(NCORES):
        out[c * sh : (c + 1) * sh, 0] = res_b.results[c]["y"][0, :sh]
    return out
